# revision 1
# baseline (speedup 1.0000x reference)
"""BinASPP Trainium2 kernel (Bass/Tile), SPMD over 8 NeuronCores.

Strategy
--------
Data-parallel over batch: N=8 images -> 1 image per core.  binarize() forward
== sign(), so every conv is a matmul over {-1,0,+1} values: exact in fp8e4,
with exact integer accumulation in fp32 PSUM.  A dilated 3x3 conv is 9
shifted 1x1 convs (taps) over a zero-padded sign image resident in SBUF.

PE efficiency:
 - the padded sign image is stored k-interleaved [128, 2, pad_image] fp8 so a
   single DoubleRow matmul contracts all K=256 input channels at once;
 - moving-operand tiles are contiguous runs of 5 *padded* rows (N=440): a
   strided multi-dim rhs AP costs ~50 PE cycles per row break (measured
   381ns vs 213ns theoretical for [8x64] tiles), so we stream whole padded
   rows and discard the 24 pad columns when reading PSUM back.

Batch-norm is training-mode (batch statistics), so per-branch per-channel
[sum, sumsq] partials are AllReduced across the 8 cores.  The collectives are
issued per branch, so each branch's BN apply (s += a_c * clip(y, lo_c, hi_c),
lo/hi = (-/+1 - d_c)/a_c, d_c offsets deferred to a final += s0 pass) runs on
DVE underneath the next branches' matmul stream; only the last branch's
allreduce+apply sits in the tail.  Branch sums ride free on the PSUM->SBUF
fp16 copy (ACT accum_out); sumsq is one DVE pass per tile (accum_out).
Branch outputs y (even integers, |y| <= 2304) are staged in SBUF as fp16
(exact), so pass 2 needs no recompute and no DRAM round trip.
"""

import numpy as np
import ml_dtypes
from contextlib import ExitStack

import concourse.bass as bass
import concourse.bacc as bacc
import concourse.mybir as mybir
import concourse.tile as tile
from concourse.bass_utils import run_bass_kernel_spmd

AF = mybir.ActivationFunctionType
ALU = mybir.AluOpType
AX = mybir.AxisListType
F32 = mybir.dt.float32
BF16 = mybir.dt.bfloat16
F16 = mybir.dt.float16
FP8 = mybir.dt.float8e4
DR = mybir.MatmulPerfMode.DoubleRow

P = 128
CIN = 256
COUT = 256
H = W = 64
HW = H * W
PAD = 12                      # max dilation rate
PH = PW = H + 2 * PAD         # 88
GUARD = 16                    # fp8 guard elements before/after each image
ILEN = GUARD + PH * PW + GUARD  # 7776 (multiple of 16 -> DR stride rule)
RATES = (1, 4, 8, 12)
NT = 2 + 9 * len(RATES)       # 38 tap matrices: pool, 1x1, 4 branches x 9
EPS = 1e-5
N_CORES = 8
# pass-1 spatial tiles: runs of full padded rows, 5 rows (440 cols) per tile
ROWTILES = [(5 * t, 5) for t in range(12)] + [(60, 4)]
NRT = len(ROWTILES)           # 13
# pass-2 tiles: 8 output rows each
RT2 = 8
NT2 = H // RT2                # 8


def build(n_cores: int = N_CORES):
    nc = bacc.Bacc(
        "TRN2",
        target_bir_lowering=False,
        debug=False,
        enable_asserts=False,
        num_devices=n_cores,
    )
    xs = nc.dram_tensor("xs", [CIN, H, W], F32, kind="ExternalInput")
    wt = nc.dram_tensor("wt", [P, NT * 2, 2, P], FP8, kind="ExternalInput")
    coef = nc.dram_tensor("coef", [P, 2, 12], F32, kind="ExternalInput")
    out = nc.dram_tensor("out", [COUT, H, W], F32, kind="ExternalOutput")

    with tile.TileContext(nc) as tc, ExitStack() as ctx:
        const = ctx.enter_context(tc.tile_pool(name="const", bufs=1))
        xload = ctx.enter_context(tc.tile_pool(name="xload", bufs=2))
        ppool = ctx.enter_context(
            tc.tile_pool(name="ppool", bufs=6, space=bass.MemorySpace.PSUM))
        psmall = ctx.enter_context(
            tc.tile_pool(name="psmall", bufs=2, space=bass.MemorySpace.PSUM))
        ybuf = ctx.enter_context(tc.tile_pool(name="ybuf", bufs=1))
        stat = ctx.enter_context(tc.tile_pool(name="stat", bufs=1))
        tmp = ctx.enter_context(tc.tile_pool(name="tmp", bufs=4))
        sqp = ctx.enter_context(tc.tile_pool(name="sqp", bufs=4))
        sbout = ctx.enter_context(tc.tile_pool(name="sbout", bufs=4))
        dram = ctx.enter_context(
            tc.tile_pool(name="dram", bufs=1, space=bass.MemorySpace.DRAM))

        # ---- weights + bn coefficient load
        lhsT = const.tile([P, NT * 2, 2, P], FP8, tag="lhsT")
        nc.sync.dma_start(lhsT[:], wt.ap())
        coef_sb = const.tile([P, 2, 12], F32, tag="coef")
        nc.sync.dma_start(coef_sb[:], coef.ap())
        epsb = stat.tile([P, 1], F32, tag="epsb")
        nc.vector.memset(epsb[:], EPS)

        def wdr(blk):
            return lhsT[:, blk]          # [P, 2, P] fp8, k-interleaved

        # ---- x -> padded k-interleaved sign image (fp8) + pooled sign
        sxp = const.tile([P, 2, ILEN], FP8, tag="sxp")
        for i in range(2):
            # zero only pad/guard areas (interior is overwritten by Sign):
            # front guard + top pad rows; bottom pad rows + back guard; and the
            # 2*PAD-wide seams (right pad of row r | left pad of row r+1).
            nc.gpsimd.memset(sxp[:, i, 0:GUARD + PAD * PW], 0.0)
            nc.gpsimd.memset(sxp[:, i, GUARD + (PAD + H) * PW:ILEN], 0.0)
            off0 = GUARD + PAD * PW - PAD
            seams = sxp[:, i, off0:off0 + (H + 1) * PW].rearrange(
                "p (r c) -> p r c", c=PW)[:, :, 0:2 * PAD]
            nc.gpsimd.memset(seams, 0.0)
        spool = const.tile([P, 2, 16], FP8, tag="spool")  # 16-wide: DR step rule
        for kc in range(2):
            xsb = xload.tile([P, H, W], F32, tag="xsb")
            nc.sync.dma_start(xsb[:], xs.ap()[kc * P:(kc + 1) * P])
            xsum = xload.tile([P, 1], F32, tag="xsum")
            nc.vector.reduce_sum(xsum[:], xsb[:], axis=AX.XY)
            nc.scalar.activation(spool[:, kc, 0:1], xsum[:], AF.Sign)
            interior = sxp[:, kc, GUARD:GUARD + PH * PW].rearrange(
                "p (r c) -> p r c", c=PW)[:, PAD:PAD + H, PAD:PAD + W]
            nc.scalar.activation(interior, xsb[:], AF.Sign)

        # s accumulator (filled branch by branch)
        s_all = const.tile([P, 2, HW], F32, tag="s_all")

        # per-branch coefficient state
        s0 = {}
        for mc in range(2):
            s0[mc] = stat.tile([P, 1], F32, tag=f"s0_{mc}", name=f"s0_{mc}")

        def allreduce_stats(j, stats_j):
            st_in = dram.tile([P, 2, 2], F32, tag=f"st_in{j}")
            st_out = dram.tile([P, 2, 2], F32, tag=f"st_out{j}",
                               addr_space="Shared" if n_cores > 4 else "Local")
            nc.sync.dma_start(st_in[:], stats_j[:])
            nc.gpsimd.collective_compute(
                "AllReduce", ALU.add,
                replica_groups=[list(range(n_cores))],
                ins=[st_in[:].opt()], outs=[st_out[:].opt()],
            )
            ar = stat.tile([P, 2, 2], F32, tag=f"stats_ar{j}")
            nc.sync.dma_start(ar[:], st_out[:])
            return ar

        def coef_j(j, mc, ar):
            """a, d from allreduced [sum, sumsq]; returns (a_, d_)."""
            cntj = float(n_cores) if j == 0 else float(HW * n_cores)
            mu = tmp.tile([P, 1], F32, tag="mu")
            nc.vector.tensor_scalar(mu[:], ar[:, mc, 0:1], 1.0 / cntj, None,
                                    op0=ALU.mult)
            ex2 = tmp.tile([P, 1], F32, tag="ex2")
            nc.vector.tensor_scalar(ex2[:], ar[:, mc, 1:2], 1.0 / cntj, None,
                                    op0=ALU.mult)
            var = tmp.tile([P, 1], F32, tag="var")
            nc.vector.tensor_tensor(var[:], mu[:], mu[:], op=ALU.mult)
            nc.vector.tensor_tensor(var[:], ex2[:], var[:], op=ALU.subtract)
            std = tmp.tile([P, 1], F32, tag="std")
            nc.scalar.activation(std[:], var[:], AF.Sqrt, bias=epsb[:])
            inv = tmp.tile([P, 1], F32, tag="inv")
            nc.vector.reciprocal(inv[:], std[:])
            a_ = stat.tile([P, 1], F32, tag=f"a{j}_{mc}")
            nc.vector.tensor_tensor(a_[:], coef_sb[:, mc, 2 * j:2 * j + 1],
                                    inv[:], op=ALU.mult)
            d_ = tmp.tile([P, 1], F32, tag="d")
            nc.vector.tensor_tensor(d_[:], mu[:], a_[:], op=ALU.mult)
            nc.vector.tensor_tensor(d_[:], coef_sb[:, mc, 2 * j + 1:2 * j + 2],
                                    d_[:], op=ALU.subtract)
            return a_, d_

        # ---- pool branch (j=0): y_pool = sign(W_pool) @ sign(mean(x))
        stats_0 = stat.tile([P, 2, 2], F32, tag="stats_0")
        ypool = []
        for mc in range(2):
            yp = psmall.tile([P, 1], F32, tag="yp")
            nc.tensor.matmul(yp[:], wdr(0 * 2 + mc), spool[:, :, 0:1],
                             start=True, stop=True, perf_mode=DR)
            ys = stat.tile([P, 1], F32, tag=f"ypool{mc}")
            nc.scalar.activation(ys[:], yp[:], AF.Copy)
            nc.vector.tensor_copy(stats_0[:, mc, 0:1], ys[:])
            nc.vector.tensor_tensor(stats_0[:, mc, 1:2], ys[:], ys[:], op=ALU.mult)
            ypool.append(ys)
        ar0 = allreduce_stats(0, stats_0)
        for mc in range(2):
            a_, d_ = coef_j(0, mc, ar0)
            nc.vector.tensor_tensor(s0[mc][:], a_[:], ypool[mc][:], op=ALU.mult)
            nc.vector.tensor_tensor(s0[mc][:], s0[mc][:], d_[:], op=ALU.add)

        # ---- conv branches (j=1 the 1x1, j=2..5 the dilated 3x3s)
        branches = [(1, None)] + [(2 + 9 * i, r) for i, r in enumerate(RATES)]
        for j, (tap0, r) in enumerate(branches, start=1):
            if r is None:
                taps = [(tap0, 1, 1)]
            else:
                taps = [(tap0 + 3 * ky + kx, ky, kx)
                        for ky in range(3) for kx in range(3)]
            stats_j = stat.tile([P, 2, 2], F32, tag=f"stats_{j}")
            y16 = {}
            for mc in range(2):
                yt = ybuf.tile([P, HW], F16, tag=f"y{j}_{mc}")
                y16[mc] = yt
                sum_p = stat.tile([P, NRT], F32, tag=f"sump{j}_{mc}")
                sq_p = stat.tile([P, NRT], F32, tag=f"sqp{j}_{mc}")
                for it, (h0, nr) in enumerate(ROWTILES):
                    n = nr * PW
                    acc = ppool.tile([P, 5 * PW], F32, tag="acc")
                    for i_mm, (tap, ky, kx) in enumerate(taps):
                        rr = 0 if r is None else r
                        pos = GUARD + (PAD + h0 + rr * (ky - 1)) * PW + rr * (kx - 1)
                        rhs = sxp[:, :, pos:pos + n]
                        nc.tensor.matmul(acc[:, 0:n], wdr(tap * 2 + mc), rhs,
                                         start=(i_mm == 0),
                                         stop=(i_mm == len(taps) - 1),
                                         perf_mode=DR)
                    acc3 = acc[:, 0:n].rearrange("p (r c) -> p r c", c=PW)
                    useful = acc3[:, :, PAD:PAD + W]
                    ysl = yt[:, h0 * W:(h0 + nr) * W]
                    nc.scalar.activation(ysl, useful, AF.Copy,
                                         accum_out=sum_p[:, it:it + 1])
                    sqt = sqp.tile([P, 5 * W], F32, tag="sqt")
                    nc.scalar.activation(sqt[:, 0:nr * W], ysl, AF.Square,
                                         accum_out=sq_p[:, it:it + 1])
                # off the DVE queue: a stalled apply (waiting on a previous
                # branch's allreduce) must not delay this branch's stats
                red = sqp.tile([P, NRT], F32, tag="red")
                nc.scalar.activation(red[:], sum_p[:], AF.Copy,
                                     accum_out=stats_j[:, mc, 0:1])
                nc.scalar.activation(red[:], sq_p[:], AF.Copy,
                                     accum_out=stats_j[:, mc, 1:2])
            arj = allreduce_stats(j, stats_j)
            for mc in range(2):
                a_, d_ = coef_j(j, mc, arj)
                inva = tmp.tile([P, 1], F32, tag="inva")
                nc.vector.reciprocal(inva[:], a_[:])
                lo = stat.tile([P, 1], F32, tag=f"lo{j}_{mc}")
                nc.vector.tensor_scalar(lo[:], d_[:], -1.0, -1.0,
                                        op0=ALU.mult, op1=ALU.add)
                nc.vector.tensor_tensor(lo[:], lo[:], inva[:], op=ALU.mult)
                hi = stat.tile([P, 1], F32, tag=f"hi{j}_{mc}")
                nc.vector.tensor_scalar(hi[:], d_[:], -1.0, 1.0,
                                        op0=ALU.mult, op1=ALU.add)
                nc.vector.tensor_tensor(hi[:], hi[:], inva[:], op=ALU.mult)
                nc.vector.tensor_tensor(s0[mc][:], s0[mc][:], d_[:], op=ALU.add)
                # apply branch j on DVE (overlaps later branches' matmuls)
                for t in range(NT2):
                    ssl = s_all[:, mc, t * 512:(t + 1) * 512]
                    u = sbout.tile([P, 512], F32, tag="u")
                    nc.vector.tensor_scalar(u[:], y16[mc][:, t * 512:(t + 1) * 512],
                                            lo[:], hi[:], op0=ALU.max, op1=ALU.min)
                    if j == 1:
                        nc.vector.tensor_scalar(ssl, u[:], a_[:], None,
                                                op0=ALU.mult)
                    else:
                        nc.vector.scalar_tensor_tensor(ssl, u[:], a_[:], ssl,
                                                       op0=ALU.mult, op1=ALU.add)

        # ---- final: add s0 (pool value + all BN offsets), store
        for mc in range(2):
            for t in range(NT2):
                sf = sbout.tile([P, RT2, W], F32, tag="sf")
                nc.vector.tensor_scalar(sf[:], s_all[:, mc, t * 512:(t + 1) * 512],
                                        1.0, s0[mc][:], op0=ALU.mult, op1=ALU.add)
                nc.sync.dma_start(
                    out.ap()[mc * P:(mc + 1) * P, t * RT2:(t + 1) * RT2, :], sf[:])

    nc.compile()
    return nc


def pack_weights(w_pool, w1, w3):
    """Host filter transform: sign -> DoubleRow k-interleave, fp8.

    wt[k, t*2+mc, i, m] = sign(W_t[mc*128+m, i*128+k]); block (t*2+mc) is the
    stationary [2, 128] operand for logical tap t / out-channel chunk mc.
    """
    mats = [np.sign(np.asarray(w_pool, np.float32).reshape(COUT, CIN)),
            np.sign(np.asarray(w1, np.float32).reshape(COUT, CIN))]
    w3 = np.asarray(w3, np.float32)
    for i in range(len(RATES)):
        for ky in range(3):
            for kx in range(3):
                mats.append(np.sign(w3[i, :, :, ky, kx]))
    wt = np.zeros((P, NT * 2, 2, P), np.float32)  # [k, blk, i, m]
    for t, m in enumerate(mats):
        for mc in range(2):
            for i in range(2):
                blk = m[mc * P:(mc + 1) * P, i * P:(i + 1) * P]   # [m, k]
                wt[:, t * 2 + mc, i, :] = blk.T
    return wt.astype(mybir.dt.np(FP8))


def pack_coef(g_pool, b_pool, g1, b1, g3, b3):
    gs = [g_pool, g1] + [g3[i] for i in range(len(RATES))]
    bs = [b_pool, b1] + [b3[i] for i in range(len(RATES))]
    coef = np.zeros((P, 2, 12), np.float32)
    for j in range(6):
        g = np.asarray(gs[j], np.float32)
        b = np.asarray(bs[j], np.float32)
        for mc in range(2):
            coef[:, mc, 2 * j] = g[mc * P:(mc + 1) * P]
            coef[:, mc, 2 * j + 1] = b[mc * P:(mc + 1) * P]
    return coef


_NC = None


def _get_nc():
    global _NC
    if _NC is None:
        _NC = build(N_CORES)
    return _NC


def make_in_maps(x, w_pool, g_pool, b_pool, w1, g1, b1, w3, g3, b3):
    x = np.asarray(x, np.float32)
    wt = pack_weights(w_pool, w1, w3)
    coef = pack_coef(g_pool, b_pool, g1, b1, g3, b3)
    return [
        {"xs": np.ascontiguousarray(x[c]), "wt": wt, "coef": coef}
        for c in range(x.shape[0])
    ]


def kernel(x, w_pool, g_pool, b_pool, w1, g1, b1, w3, g3, b3):
    nc = _get_nc()
    in_maps = make_in_maps(x, w_pool, g_pool, b_pool, w1, g1, b1, w3, g3, b3)
    res = run_bass_kernel_spmd(nc, in_maps, core_ids=list(range(N_CORES)))
    return np.stack([res.results[c]["out"] for c in range(N_CORES)], axis=0)



# revision 4
# speedup vs baseline: 2.5379x; 2.5379x over previous
"""BinASPP Trainium2 kernel (Bass/Tile), SPMD over 8 NeuronCores.

Strategy (v2)
-------------
Data-parallel over batch: N=8 images -> 1 image per core.  binarize() forward
== sign(), so every conv is a matmul over {-1,+1} values: exact in fp8e4 with
integer accumulation in fp32 PSUM.  A dilated 3x3 conv is 9 shifted 1x1 convs
(taps) over a zero-padded sign image resident in SBUF; a DoubleRow matmul
contracts all K=256 input channels at once, streaming contiguous runs of 5
padded rows (440 cols) per PSUM bank.

Engine split (the v1 bottlenecks were ACT at 100% and 6 serialized 28us
AllReduces):
 - ACT: plain PSUM->SBUF fp16 copies of the useful columns (no accum_out: the
   187ns accumulator-read aux per op is gone).  y fp16 is exact (even ints,
   |y| <= 2304).
 - DVE: per-branch BN stats via bn_stats (8x512 chunks) + bn_aggr ->
   (mean, var); clip pass in fp16 (4x DVE mode).
 - Pool: s += a*clip(y) multiply-add (scalar_tensor_tensor measures 0.83
   ns/elem there), plus the tiny cross-core merge trees.
 - Sync-BN: per-branch-group AllGather (15us, vs AllReduce's 28.1us) of
   (mean, var) pairs; each core merges the 8 cores' moments locally
   (var_g = E[var] + E[mean^2] - E[mean]^2).  Four groups pipeline under the
   matmul stream so only the last branch's gather sits in the tail.
BN offsets d_j fold into a per-channel s0 added in the final pass.
"""

import numpy as np
import ml_dtypes
from contextlib import ExitStack

import concourse.bass as bass
import concourse.bacc as bacc
import concourse.mybir as mybir
import concourse.tile as tile
from concourse.bass_utils import run_bass_kernel_spmd

AF = mybir.ActivationFunctionType
ALU = mybir.AluOpType
AX = mybir.AxisListType
F32 = mybir.dt.float32
F16 = mybir.dt.float16
FP8 = mybir.dt.float8e4
DR = mybir.MatmulPerfMode.DoubleRow

P = 128
CIN = 256
COUT = 256
H = W = 64
HW = H * W
PAD = 12                      # max dilation rate
PH = PW = H + 2 * PAD         # 88
GUARD = 16                    # fp8 guard elements before/after each image
ILEN = GUARD + PH * PW + GUARD  # 7776 (multiple of 16 -> DR stride rule)
RATES = (1, 4, 8, 12)
NT = 2 + 9 * len(RATES)       # 38 tap matrices: pool, 1x1, 4 branches x 9
EPS = 1e-5
N_CORES = 8
# spatial tiles: runs of full padded rows, 5 rows (440 cols) per PSUM bank
ROWTILES = [(5 * t, 5) for t in range(12)] + [(60, 4)]
NRT = len(ROWTILES)           # 13
NCHUNK = 8                    # bn_stats chunks of 512 over HW=4096
# branch ids: 0=pool, 1=1x1, 2=r1, 3=r4, 4=r8, 5=r12 (coef layout order).
# Section emission order: r1 first (matmuls start as soon as its rows are
# signed); the 1x1's 26 tiles are interleaved INTO the r4 section so its
# ~12us of PSUM drains never burst the ACT queue; r12 last so its gather is
# the only tail collective.  Each inner list is one emission unit.
SECTIONS = [[(2, 2, 1)], [(3, 11, 4), (1, 1, None)], [(4, 20, 8)],
            [(5, 29, 12)]]
# branch groups for the stats AllGathers (stats-completion order)
GROUPS = [[0, 2], [3], [1, 4], [5]]


def build(n_cores: int = N_CORES):
    nc = bacc.Bacc(
        "TRN2",
        target_bir_lowering=False,
        debug=False,
        enable_asserts=False,
        num_devices=n_cores,
    )
    xs = nc.dram_tensor("xs", [CIN, H, W], F32, kind="ExternalInput")
    wt = nc.dram_tensor("wt", [P, NT * 2, 2, P], FP8, kind="ExternalInput")
    coef = nc.dram_tensor("coef", [P, 2, 12], F32, kind="ExternalInput")
    out = nc.dram_tensor("out", [COUT, H, W], F16, kind="ExternalOutput")

    with tile.TileContext(nc) as tc, ExitStack() as ctx:
        const = ctx.enter_context(tc.tile_pool(name="const", bufs=1))
        xload = ctx.enter_context(tc.tile_pool(name="xload", bufs=2))
        ppool = ctx.enter_context(
            tc.tile_pool(name="ppool", bufs=6, space=bass.MemorySpace.PSUM))
        psmall = ctx.enter_context(
            tc.tile_pool(name="psmall", bufs=2, space=bass.MemorySpace.PSUM))
        ybuf = ctx.enter_context(tc.tile_pool(name="ybuf", bufs=1))
        ubuf = ctx.enter_context(tc.tile_pool(name="ubuf", bufs=2))
        stat = ctx.enter_context(tc.tile_pool(name="stat", bufs=1))
        tmp = ctx.enter_context(tc.tile_pool(name="tmp", bufs=4))
        sbout = ctx.enter_context(tc.tile_pool(name="sbout", bufs=2))
        dram = ctx.enter_context(
            tc.tile_pool(name="dram", bufs=1, space=bass.MemorySpace.DRAM))

        # ---- weights + bn coefficient load.  DMA transfers serialize on the
        # global DMA engines, so order by need: taps for pool/1x1/r1 first,
        # then the x image chunks, then the remaining taps (r4/r8/r12, not
        # needed until ~40us in).
        NBLK1 = 22            # blocks 0..21: pool, 1x1, r1 taps
        lhsT = const.tile([P, NT * 2, 2, P], FP8, tag="lhsT")
        nc.sync.dma_start(lhsT[:, 0:NBLK1], wt.ap()[:, 0:NBLK1])
        coef_sb = const.tile([P, 2, 12], F32, tag="coef")
        nc.scalar.dma_start(coef_sb[:], coef.ap())

        def wdr(blk):
            return lhsT[:, blk]          # [P, 2, P] fp8, k-interleaved

        # ---- x -> padded k-interleaved sign image (fp8) + pooled sign
        sxp = const.tile([P, 2, ILEN], FP8, tag="sxp")
        for i in range(2):
            # zero only pad/guard areas (interior is overwritten by Sign)
            nc.gpsimd.memset(sxp[:, i, 0:GUARD + PAD * PW], 0.0)
            nc.gpsimd.memset(sxp[:, i, GUARD + (PAD + H) * PW:ILEN], 0.0)
            off0 = GUARD + PAD * PW - PAD
            seams = sxp[:, i, off0:off0 + (H + 1) * PW].rearrange(
                "p (r c) -> p r c", c=PW)[:, :, 0:2 * PAD]
            nc.gpsimd.memset(seams, 0.0)
        # s accumulator zero-fill (Pool is idle at t=0)
        s_all = const.tile([P, 2, HW], F32, tag="s_all")
        for mc in range(2):
            nc.gpsimd.memset(s_all[:, mc], 0.0)

        spool = const.tile([P, 2, 16], FP8, tag="spool")  # 16-wide: DR step rule
        XBLK = 4
        XR = H // XBLK            # 16 rows per block
        interiors = [
            sxp[:, kc, GUARD:GUARD + PH * PW].rearrange(
                "p (r c) -> p r c", c=PW)[:, PAD:PAD + H, PAD:PAD + W]
            for kc in range(2)]
        xs4 = {kc: xload.tile([P, XBLK], F32, tag=f"xs4_{kc}",
                              name=f"xs4_{kc}") for kc in range(2)}
        # interleave kc within each row-block so early rows of BOTH k-chunks
        # are signed first (matmuls contract both chunks)
        for b in range(XBLK):
            for kc in range(2):
                xsb = xload.tile([P, XR, W], F32, tag="xsb")
                eng = nc.sync if kc == 0 else nc.scalar
                eng.dma_start(
                    xsb[:], xs.ap()[kc * P:(kc + 1) * P, b * XR:(b + 1) * XR])
                nc.vector.reduce_sum(xs4[kc][:, b:b + 1], xsb[:], axis=AX.XY)
                nc.scalar.activation(
                    interiors[kc][:, b * XR:(b + 1) * XR], xsb[:], AF.Sign)
        for kc in range(2):
            xsum = xload.tile([P, 1], F32, tag=f"xsum_{kc}",
                              name=f"xsum_{kc}")
            nc.vector.reduce_sum(xsum[:], xs4[kc][:], axis=AX.X)
            nc.scalar.activation(spool[:, kc, 0:1], xsum[:], AF.Sign)
        # remaining taps (r4/r8/r12) after the x image is on its way
        nc.sync.dma_start(lhsT[:, NBLK1:], wt.ap()[:, NBLK1:])

        # per-branch-group stats tiles: [P, G, 2mc, (mean|var)]
        stats_g = {}
        for g, members in enumerate(GROUPS):
            stats_g[g] = stat.tile([P, len(members), 2, 2], F32,
                                   tag=f"stats_g{g}", name=f"stats_g{g}")
        # deferred per-channel offset (sum of d_j + pool-branch value)
        s0 = {}
        for mc in range(2):
            s0[mc] = stat.tile([P, 1], F32, tag=f"s0_{mc}", name=f"s0_{mc}")
            nc.vector.memset(s0[mc][:], 0.0)

        # ---- pool branch (j=0): y_pool = sign(W_pool) @ sign(mean(x)).
        # Emitted AFTER the first conv section so its wait on spool (ready
        # only once the whole image is loaded) doesn't head-of-line block the
        # PE queue in front of the r1 matmuls.
        ypool = {}

        def emit_pool_branch():
            for mc in range(2):
                yp = psmall.tile([P, 1], F32, tag="yp")
                nc.tensor.matmul(yp[:], wdr(0 * 2 + mc), spool[:, :, 0:1],
                                 start=True, stop=True, perf_mode=DR)
                ys = stat.tile([P, 1], F32, tag=f"ypool{mc}",
                               name=f"ypool{mc}")
                nc.scalar.activation(ys[:], yp[:], AF.Copy)
                ypool[mc] = ys
                # stats: mean = y (count 1 per core), var = 0
                nc.vector.tensor_copy(stats_g[0][:, 0, mc, 0:1], ys[:])
                nc.vector.memset(stats_g[0][:, 0, mc, 1:2], 0.0)

        # ---- group coef state (filled by issue/merge closures)
        coefs = {}   # g -> dict(a=, lo=, hi=) tiles [P, 2, G]
        y16_all = {}  # branch j -> {mc: y16 tile}

        def issue_gather(g):
            members = GROUPS[g]
            G = len(members)
            st_in = dram.tile([P, G * 4], F32, tag=f"st_in{g}")
            st_out = dram.tile([8, P, G * 4], F32, tag=f"st_out{g}",
                               addr_space="Shared" if n_cores > 4 else "Local")
            nc.sync.dma_start(st_in[:], stats_g[g][:])
            nc.gpsimd.collective_compute(
                "AllGather", ALU.bypass,
                replica_groups=[list(range(n_cores))],
                ins=[st_in[:].opt()], outs=[st_out[:].opt()],
            )
            gath = stat.tile([P, 8, G * 4], F32, tag=f"gath{g}",
                             name=f"gath{g}")
            nc.sync.dma_start(gath[:], st_out[:].rearrange("c p f -> p c f"))
            return gath

        def merge_and_coefs(g, gath):
            """Cross-core moment merge (Pool trees) + per-branch coefs (DVE)."""
            members = GROUPS[g]
            G = len(members)
            F = G * 4
            # Pool: tree-reduce the 8 cores' (mean, var) pairs; also squares
            # of everything (we use only the mean^2 lanes).
            t4 = tmp.tile([P, 4, F], F32, tag=f"t4_{g}", name=f"t4_{g}")
            nc.gpsimd.tensor_tensor(t4[:], gath[:, 0:4], gath[:, 4:8],
                                    op=ALU.add)
            t2 = tmp.tile([P, 2, F], F32, tag=f"t2_{g}", name=f"t2_{g}")
            nc.gpsimd.tensor_tensor(t2[:], t4[:, 0:2], t4[:, 2:4], op=ALU.add)
            t1 = tmp.tile([P, F], F32, tag=f"t1_{g}", name=f"t1_{g}")
            nc.gpsimd.tensor_tensor(t1[:], t2[:, 0], t2[:, 1], op=ALU.add)
            sq = tmp.tile([P, 8, F], F32, tag=f"sq_{g}", name=f"sq_{g}")
            nc.gpsimd.tensor_tensor(sq[:], gath[:], gath[:], op=ALU.mult)
            q4 = tmp.tile([P, 4, F], F32, tag=f"q4_{g}", name=f"q4_{g}")
            nc.gpsimd.tensor_tensor(q4[:], sq[:, 0:4], sq[:, 4:8], op=ALU.add)
            q2 = tmp.tile([P, 2, F], F32, tag=f"q2_{g}", name=f"q2_{g}")
            nc.gpsimd.tensor_tensor(q2[:], q4[:, 0:2], q4[:, 2:4], op=ALU.add)
            q1 = tmp.tile([P, F], F32, tag=f"q1_{g}", name=f"q1_{g}")
            nc.gpsimd.tensor_tensor(q1[:], q2[:, 0], q2[:, 1], op=ALU.add)

            # views: t1/q1 layout [(gi, mc, field)] with field stride 1
            t1v = t1[:].rearrange("p (g m f) -> p g m f", m=2, f=2)
            q1v = q1[:].rearrange("p (g m f) -> p g m f", m=2, f=2)
            a_t = stat.tile([P, 2, G], F32, tag=f"a{g}", name=f"a{g}")
            lo_t = stat.tile([P, 2, G], F32, tag=f"lo{g}", name=f"lo{g}")
            hi_t = stat.tile([P, 2, G], F32, tag=f"hi{g}", name=f"hi{g}")
            for mc in range(2):
                means = t1v[:, :, mc, 0]           # [P, G] Sum_c mean_c
                vars_ = t1v[:, :, mc, 1]           # [P, G] Sum_c var_c
                msq = q1v[:, :, mc, 0]             # [P, G] Sum_c mean_c^2
                mu = tmp.tile([P, G], F32, tag=f"mu{g}", name=f"mu{g}")
                nc.vector.tensor_scalar(mu[:], means, 1.0 / n_cores, None,
                                        op0=ALU.mult)
                # ex2c = E_c[var] + E_c[mean^2]
                ex2 = tmp.tile([P, G], F32, tag=f"ex2{g}", name=f"ex2{g}")
                nc.vector.tensor_tensor(ex2[:], vars_, msq, op=ALU.add)
                nc.vector.tensor_scalar(ex2[:], ex2[:], 1.0 / n_cores, None,
                                        op0=ALU.mult)
                var = tmp.tile([P, G], F32, tag=f"var{g}", name=f"var{g}")
                nc.vector.tensor_tensor(var[:], mu[:], mu[:], op=ALU.mult)
                nc.vector.tensor_tensor(var[:], ex2[:], var[:],
                                        op=ALU.subtract)
                nc.vector.tensor_scalar(var[:], var[:], EPS, None, op0=ALU.add)
                std = tmp.tile([P, G], F32, tag=f"std{g}", name=f"std{g}")
                nc.scalar.activation(std[:], var[:], AF.Sqrt)
                inv = tmp.tile([P, G], F32, tag=f"inv{g}", name=f"inv{g}")
                nc.vector.reciprocal(inv[:], std[:])
                # a = gamma * inv;  d = beta - a*mu
                gsl = [coef_sb[:, mc, 2 * j:2 * j + 1] for j in members]
                bsl = [coef_sb[:, mc, 2 * j + 1:2 * j + 2] for j in members]
                a_ = a_t[:, mc]
                d_ = tmp.tile([P, G], F32, tag=f"d{g}", name=f"d{g}")
                for gi in range(len(members)):
                    nc.vector.tensor_tensor(a_[:, gi:gi + 1], gsl[gi],
                                            inv[:, gi:gi + 1], op=ALU.mult)
                    nc.vector.tensor_tensor(d_[:, gi:gi + 1], mu[:, gi:gi + 1],
                                            a_[:, gi:gi + 1], op=ALU.mult)
                    nc.vector.tensor_tensor(d_[:, gi:gi + 1], bsl[gi],
                                            d_[:, gi:gi + 1], op=ALU.subtract)
                inva = tmp.tile([P, G], F32, tag=f"inva{g}", name=f"inva{g}")
                nc.vector.reciprocal(inva[:], a_[:])
                # lo = (-1 - d)/a ; hi = (1 - d)/a
                nc.vector.tensor_scalar(lo_t[:, mc], d_[:], -1.0, -1.0,
                                        op0=ALU.mult, op1=ALU.add)
                nc.vector.tensor_tensor(lo_t[:, mc], lo_t[:, mc], inva[:],
                                        op=ALU.mult)
                nc.vector.tensor_scalar(hi_t[:, mc], d_[:], -1.0, 1.0,
                                        op0=ALU.mult, op1=ALU.add)
                nc.vector.tensor_tensor(hi_t[:, mc], hi_t[:, mc], inva[:],
                                        op=ALU.mult)
                # s0 += sum_j d_j  (+ a_pool*ypool for group 0)
                dsum = tmp.tile([P, 1], F32, tag="dsum")
                if G > 1:
                    nc.vector.reduce_sum(dsum[:], d_[:], axis=AX.X)
                else:
                    nc.vector.tensor_copy(dsum[:], d_[:])
                nc.vector.tensor_tensor(s0[mc][:], s0[mc][:], dsum[:],
                                        op=ALU.add)
                if g == 0:
                    ay = tmp.tile([P, 1], F32, tag="ay")
                    nc.vector.tensor_tensor(ay[:], a_[:, 0:1], ypool[mc][:],
                                            op=ALU.mult)
                    nc.vector.tensor_tensor(s0[mc][:], s0[mc][:], ay[:],
                                            op=ALU.add)
            coefs[g] = dict(a=a_t, lo=lo_t, hi=hi_t)

        def apply_branch(g, gi, j, halves=1):
            """clip on DVE (fp16 4x, cheap there); s += a*u on Pool — the DVE
            queue must stay prompt for the stream's bn_stats.  halves>1 splits
            each mad so tail finals can start per-half."""
            c = coefs[g]
            hn = HW // halves
            for mc in range(2):
                yt = y16_all[j][mc]
                # w = a*clip(y, lo, hi): two DVE fp16 passes (4x mode).
                # scalar_tensor_tensor is not a legal Pool opcode on TRN2, so
                # the accumulate is a plain Pool tensor_tensor add.
                u = ubuf.tile([P, HW], F16, tag="u")
                nc.vector.tensor_scalar(u[:], yt[:],
                                        c["lo"][:, mc, gi:gi + 1],
                                        c["hi"][:, mc, gi:gi + 1],
                                        op0=ALU.max, op1=ALU.min)
                nc.vector.tensor_scalar(u[:], u[:], c["a"][:, mc, gi:gi + 1],
                                        None, op0=ALU.mult)
                for h in range(halves):
                    sl = slice(h * hn, (h + 1) * hn)
                    nc.gpsimd.tensor_tensor(s_all[:, mc, sl], s_all[:, mc, sl],
                                            u[:, sl], op=ALU.add)

        # group bookkeeping: branch j -> (g, gi); stats slot for conv branches
        slot = {}
        for g, members in enumerate(GROUPS):
            for gi, j in enumerate(members):
                slot[j] = (g, gi)

        gathers = {}
        pending = []          # [(emit_after_branch, fn)]

        def emit_tile(j, tap0, r, mc, h0, nr, yt):
            taps = ([(tap0, 1, 1)] if r is None else
                    [(tap0 + 3 * ky + kx, ky, kx)
                     for ky in range(3) for kx in range(3)])
            n = nr * PW
            acc = ppool.tile([P, 5 * PW], F32, tag="acc")
            for i_mm, (tap, ky, kx) in enumerate(taps):
                rr = 0 if r is None else r
                pos = (GUARD + (PAD + h0 + rr * (ky - 1)) * PW
                       + rr * (kx - 1))
                rhs = sxp[:, :, pos:pos + n]
                nc.tensor.matmul(acc[:, 0:n], wdr(tap * 2 + mc), rhs,
                                 start=(i_mm == 0),
                                 stop=(i_mm == len(taps) - 1),
                                 perf_mode=DR)
            acc3 = acc[:, 0:n].rearrange("p (r c) -> p r c", c=PW)
            useful = acc3[:, :, PAD:PAD + W]
            nc.scalar.activation(yt[:, h0 * W:(h0 + nr) * W], useful, AF.Copy)

        def emit_stats(j, mc, yt):
            # BN stats: 8 chunks of 512 + aggregate -> (mean, var)
            g, gi = slot[j]
            bn6 = tmp.tile([P, NCHUNK, 6], F32, tag="bn6")
            y3 = yt[:].rearrange("p (c e) -> p c e", e=512)
            for cch in range(NCHUNK):
                nc.vector.bn_stats(bn6[:, cch], y3[:, cch])
            nc.vector.bn_aggr(stats_g[g][:, gi, mc, 0:2], bn6[:])

        def emit_section(defs):
            """Emit one or more branches with their tile jobs interleaved."""
            jobs = []   # per branch: list of (j, tap0, r, mc, h0, nr, yt, last)
            for (j, tap0, r) in defs:
                y16 = {mc: ybuf.tile([P, HW], F16, tag=f"y{j}_{mc}",
                                     name=f"y{j}_{mc}") for mc in range(2)}
                y16_all[j] = y16
                lst = []
                for mc in range(2):
                    for it, (h0, nr) in enumerate(ROWTILES):
                        lst.append((j, tap0, r, mc, h0, nr, y16[mc],
                                    it == NRT - 1))
                jobs.append(lst)
            # round-robin across branches, one tile job at a time
            iters = [iter(l) for l in jobs]
            live = list(range(len(iters)))
            while live:
                for bi_ in list(live):
                    try:
                        (j, tap0, r, mc, h0, nr, yt, last) = next(iters[bi_])
                    except StopIteration:
                        live.remove(bi_)
                        continue
                    emit_tile(j, tap0, r, mc, h0, nr, yt)
                    if last:
                        emit_stats(j, mc, yt)

        def emit_group_apply(gg, halves=1):
            merge_and_coefs(gg, gathers[gg])
            for gi2, jj in enumerate(GROUPS[gg]):
                if jj != 0:
                    apply_branch(gg, gi2, jj, halves=halves)

        for bi, defs in enumerate(SECTIONS):
            emit_section(defs)
            if bi == 0:
                emit_pool_branch()
            for (j, tap0, r) in defs:
                g, gi = slot[j]
                # last member of group g? -> issue its AllGather now
                if gi == len(GROUPS[g]) - 1:
                    gathers[g] = issue_gather(g)
                    pending.append((bi + 1, g))
            # emit merges/applies whose delay has elapsed (gathers above went
            # first so a pending AllGather is never queued behind mads)
            for (when, gg) in [p for p in pending]:
                if when == bi:
                    pending.remove((when, gg))
                    emit_group_apply(gg)
        # drain remaining groups (the tail group: split mads so finals can
        # start per-half)
        for (when, gg) in pending:
            emit_group_apply(gg, halves=2)

        # ---- final: add s0 (pool value + all BN offsets), store as fp16
        for mc in range(2):
            for t in range(2):
                sf = sbout.tile([P, 2048], F16, tag="sf")
                nc.vector.tensor_scalar(sf[:],
                                        s_all[:, mc, t * 2048:(t + 1) * 2048],
                                        1.0, s0[mc][:],
                                        op0=ALU.mult, op1=ALU.add)
                eng = nc.sync if t == 0 else nc.scalar
                eng.dma_start(
                    out.ap()[mc * P:(mc + 1) * P].rearrange(
                        "m h w -> m (h w)")[:, t * 2048:(t + 1) * 2048],
                    sf[:])

    nc.compile()
    return nc


def pack_weights(w_pool, w1, w3):
    """Host filter transform: sign -> DoubleRow k-interleave, fp8.

    wt[k, t*2+mc, i, m] = sign(W_t[mc*128+m, i*128+k]); block (t*2+mc) is the
    stationary [2, 128] operand for logical tap t / out-channel chunk mc.
    """
    mats = [np.sign(np.asarray(w_pool, np.float32).reshape(COUT, CIN)),
            np.sign(np.asarray(w1, np.float32).reshape(COUT, CIN))]
    w3 = np.asarray(w3, np.float32)
    for i in range(len(RATES)):
        for ky in range(3):
            for kx in range(3):
                mats.append(np.sign(w3[i, :, :, ky, kx]))
    wt = np.zeros((P, NT * 2, 2, P), np.float32)  # [k, blk, i, m]
    for t, m in enumerate(mats):
        for mc in range(2):
            for i in range(2):
                blk = m[mc * P:(mc + 1) * P, i * P:(i + 1) * P]   # [m, k]
                wt[:, t * 2 + mc, i, :] = blk.T
    return wt.astype(mybir.dt.np(FP8))


def pack_coef(g_pool, b_pool, g1, b1, g3, b3):
    gs = [g_pool, g1] + [g3[i] for i in range(len(RATES))]
    bs = [b_pool, b1] + [b3[i] for i in range(len(RATES))]
    coef = np.zeros((P, 2, 12), np.float32)
    for j in range(6):
        g = np.asarray(gs[j], np.float32)
        b = np.asarray(bs[j], np.float32)
        for mc in range(2):
            coef[:, mc, 2 * j] = g[mc * P:(mc + 1) * P]
            coef[:, mc, 2 * j + 1] = b[mc * P:(mc + 1) * P]
    return coef


_NC = None


def _get_nc():
    global _NC
    if _NC is None:
        _NC = build(N_CORES)
    return _NC


def make_in_maps(x, w_pool, g_pool, b_pool, w1, g1, b1, w3, g3, b3):
    x = np.asarray(x, np.float32)
    wt = pack_weights(w_pool, w1, w3)
    coef = pack_coef(g_pool, b_pool, g1, b1, g3, b3)
    return [
        {"xs": np.ascontiguousarray(x[c]), "wt": wt, "coef": coef}
        for c in range(x.shape[0])
    ]


def kernel(x, w_pool, g_pool, b_pool, w1, g1, b1, w3, g3, b3):
    nc = _get_nc()
    in_maps = make_in_maps(x, w_pool, g_pool, b_pool, w1, g1, b1, w3, g3, b3)
    res = run_bass_kernel_spmd(nc, in_maps, core_ids=list(range(N_CORES)))
    return np.stack([res.results[c]["out"] for c in range(N_CORES)],
                    axis=0).astype(np.float32)


# revision 5
# speedup vs baseline: 2.7560x; 1.0860x over previous
"""BinASPP Trainium2 kernel (Bass/Tile), SPMD over 8 NeuronCores.

Strategy (v2)
-------------
Data-parallel over batch: N=8 images -> 1 image per core.  binarize() forward
== sign(), so every conv is a matmul over {-1,+1} values: exact in fp8e4 with
integer accumulation in fp32 PSUM.  A dilated 3x3 conv is 9 shifted 1x1 convs
(taps) over a zero-padded sign image resident in SBUF; a DoubleRow matmul
contracts all K=256 input channels at once, streaming contiguous runs of 5
padded rows (440 cols) per PSUM bank.

Engine split (the v1 bottlenecks were ACT at 100% and 6 serialized 28us
AllReduces):
 - ACT: plain PSUM->SBUF fp16 copies of the useful columns (no accum_out: the
   187ns accumulator-read aux per op is gone).  y fp16 is exact (even ints,
   |y| <= 2304).
 - DVE: per-branch BN stats via bn_stats (8x512 chunks) + bn_aggr ->
   (mean, var); clip pass in fp16 (4x DVE mode).
 - Pool: s += a*clip(y) multiply-add (scalar_tensor_tensor measures 0.83
   ns/elem there), plus the tiny cross-core merge trees.
 - Sync-BN: per-branch-group AllGather (15us, vs AllReduce's 28.1us) of
   (mean, var) pairs; each core merges the 8 cores' moments locally
   (var_g = E[var] + E[mean^2] - E[mean]^2).  Four groups pipeline under the
   matmul stream so only the last branch's gather sits in the tail.
BN offsets d_j fold into a per-channel s0 added in the final pass.
"""

import numpy as np
import ml_dtypes
from contextlib import ExitStack

import concourse.bass as bass
import concourse.bacc as bacc
import concourse.mybir as mybir
import concourse.tile as tile
from concourse.bass_utils import run_bass_kernel_spmd

AF = mybir.ActivationFunctionType
ALU = mybir.AluOpType
AX = mybir.AxisListType
F32 = mybir.dt.float32
F16 = mybir.dt.float16
FP8 = mybir.dt.float8e4
DR = mybir.MatmulPerfMode.DoubleRow

P = 128
CIN = 256
COUT = 256
H = W = 64
HW = H * W
PAD = 12                      # max dilation rate
PH = PW = H + 2 * PAD         # 88
GUARD = 16                    # fp8 guard elements before/after each image
ILEN = GUARD + PH * PW + GUARD  # 7776 (multiple of 16 -> DR stride rule)
RATES = (1, 4, 8, 12)
NT = 2 + 9 * len(RATES)       # 38 tap matrices: pool, 1x1, 4 branches x 9
EPS = 1e-5
N_CORES = 8
# spatial tiles: runs of full padded rows, 5 rows (440 cols) per PSUM bank
ROWTILES = [(5 * t, 5) for t in range(12)] + [(60, 4)]
NRT = len(ROWTILES)           # 13
NCHUNK = 8                    # bn_stats chunks of 512 over HW=4096
# branch ids: 0=pool, 1=1x1, 2=r1, 3=r4, 4=r8, 5=r12 (coef layout order).
# Section emission order: r1 first (matmuls start as soon as its rows are
# signed); the 1x1's 26 tiles are interleaved INTO the r4 section so its
# ~12us of PSUM drains never burst the ACT queue; r12 last so its gather is
# the only tail collective.  Each inner list is one emission unit.
SECTIONS = [[(2, 2, 1)], [(3, 11, 4), (1, 1, None)], [(4, 20, 8)],
            [(5, 29, 12)]]
# branch groups for the stats AllGathers (stats-completion order)
GROUPS = [[0, 2], [3], [1, 4], [5]]


def build(n_cores: int = N_CORES):
    nc = bacc.Bacc(
        "TRN2",
        target_bir_lowering=False,
        debug=False,
        enable_asserts=False,
        num_devices=n_cores,
    )
    xs = nc.dram_tensor("xs", [CIN, H, W], F32, kind="ExternalInput")
    wt = nc.dram_tensor("wt", [P, NT * 2, 2, P], FP8, kind="ExternalInput")
    coef = nc.dram_tensor("coef", [P, 2, 12], F32, kind="ExternalInput")
    out = nc.dram_tensor("out", [COUT, H, W], F16, kind="ExternalOutput")

    with tile.TileContext(nc) as tc, ExitStack() as ctx:
        const = ctx.enter_context(tc.tile_pool(name="const", bufs=1))
        xload = ctx.enter_context(tc.tile_pool(name="xload", bufs=2))
        ppool = ctx.enter_context(
            tc.tile_pool(name="ppool", bufs=6, space=bass.MemorySpace.PSUM))
        psmall = ctx.enter_context(
            tc.tile_pool(name="psmall", bufs=2, space=bass.MemorySpace.PSUM))
        ybuf = ctx.enter_context(tc.tile_pool(name="ybuf", bufs=1))
        stat = ctx.enter_context(tc.tile_pool(name="stat", bufs=1))
        tmp = ctx.enter_context(tc.tile_pool(name="tmp", bufs=4))
        sbout = ctx.enter_context(tc.tile_pool(name="sbout", bufs=2))
        dram = ctx.enter_context(
            tc.tile_pool(name="dram", bufs=1, space=bass.MemorySpace.DRAM))

        # ---- weights + bn coefficient load.  DMA transfers serialize on the
        # global DMA engines, so order by need: taps for pool/1x1/r1 first,
        # then the x image chunks, then the remaining taps (r4/r8/r12, not
        # needed until ~40us in).
        NBLK1 = 22            # blocks 0..21: pool, 1x1, r1 taps
        lhsT = const.tile([P, NT * 2, 2, P], FP8, tag="lhsT")
        nc.sync.dma_start(lhsT[:, 0:NBLK1], wt.ap()[:, 0:NBLK1])
        coef_sb = const.tile([P, 2, 12], F32, tag="coef")
        nc.scalar.dma_start(coef_sb[:], coef.ap())

        def wdr(blk):
            return lhsT[:, blk]          # [P, 2, P] fp8, k-interleaved

        # ---- x -> padded k-interleaved sign image (fp8) + pooled sign
        sxp = const.tile([P, 2, ILEN], FP8, tag="sxp")
        for i in range(2):
            # zero only pad/guard areas (interior is overwritten by Sign)
            nc.gpsimd.memset(sxp[:, i, 0:GUARD + PAD * PW], 0.0)
            nc.gpsimd.memset(sxp[:, i, GUARD + (PAD + H) * PW:ILEN], 0.0)
            off0 = GUARD + PAD * PW - PAD
            seams = sxp[:, i, off0:off0 + (H + 1) * PW].rearrange(
                "p (r c) -> p r c", c=PW)[:, :, 0:2 * PAD]
            nc.gpsimd.memset(seams, 0.0)
        spool = const.tile([P, 2, 16], FP8, tag="spool")  # 16-wide: DR step rule
        XBLK = 4
        XR = H // XBLK            # 16 rows per block
        interiors = [
            sxp[:, kc, GUARD:GUARD + PH * PW].rearrange(
                "p (r c) -> p r c", c=PW)[:, PAD:PAD + H, PAD:PAD + W]
            for kc in range(2)]
        xs4 = {kc: xload.tile([P, XBLK], F32, tag=f"xs4_{kc}",
                              name=f"xs4_{kc}") for kc in range(2)}
        # interleave kc within each row-block so early rows of BOTH k-chunks
        # are signed first (matmuls contract both chunks)
        for b in range(XBLK):
            for kc in range(2):
                xsb = xload.tile([P, XR, W], F32, tag="xsb")
                eng = nc.sync if kc == 0 else nc.scalar
                eng.dma_start(
                    xsb[:], xs.ap()[kc * P:(kc + 1) * P, b * XR:(b + 1) * XR])
                nc.vector.reduce_sum(xs4[kc][:, b:b + 1], xsb[:], axis=AX.XY)
                nc.scalar.activation(
                    interiors[kc][:, b * XR:(b + 1) * XR], xsb[:], AF.Sign)
        for kc in range(2):
            xsum = xload.tile([P, 1], F32, tag=f"xsum_{kc}",
                              name=f"xsum_{kc}")
            nc.vector.reduce_sum(xsum[:], xs4[kc][:], axis=AX.X)
            nc.scalar.activation(spool[:, kc, 0:1], xsum[:], AF.Sign)
        # remaining taps (r4/r8/r12) after the x image is on its way
        nc.sync.dma_start(lhsT[:, NBLK1:], wt.ap()[:, NBLK1:])

        # per-branch-group stats tiles: [P, G, 2mc, (mean|var)]
        stats_g = {}
        for g, members in enumerate(GROUPS):
            stats_g[g] = stat.tile([P, len(members), 2, 2], F32,
                                   tag=f"stats_g{g}", name=f"stats_g{g}")
        # deferred per-channel offset (sum of d_j + pool-branch value)
        s0 = stat.tile([P, 2], F32, tag="s0", name="s0")
        nc.vector.memset(s0[:], 0.0)

        # ---- pool branch (j=0): y_pool = sign(W_pool) @ sign(mean(x)).
        # Emitted AFTER the first conv section so its wait on spool (ready
        # only once the whole image is loaded) doesn't head-of-line block the
        # PE queue in front of the r1 matmuls.
        ypool = {}

        def emit_pool_branch():
            for mc in range(2):
                yp = psmall.tile([P, 1], F32, tag="yp")
                nc.tensor.matmul(yp[:], wdr(0 * 2 + mc), spool[:, :, 0:1],
                                 start=True, stop=True, perf_mode=DR)
                ys = stat.tile([P, 1], F32, tag=f"ypool{mc}",
                               name=f"ypool{mc}")
                nc.scalar.activation(ys[:], yp[:], AF.Copy)
                ypool[mc] = ys
                # stats: mean = y (count 1 per core), var = 0
                nc.vector.tensor_copy(stats_g[0][:, 0, mc, 0:1], ys[:])
                nc.vector.memset(stats_g[0][:, 0, mc, 1:2], 0.0)

        # ---- group coef state (filled by issue/merge closures)
        coefs = {}   # g -> dict(a=, lo=, hi=) tiles [P, 2, G]
        y16_all = {}  # branch j -> {mc: y16 tile}

        def issue_gather(g):
            members = GROUPS[g]
            G = len(members)
            st_in = dram.tile([P, G * 4], F32, tag=f"st_in{g}")
            st_out = dram.tile([8, P, G * 4], F32, tag=f"st_out{g}",
                               addr_space="Shared" if n_cores > 4 else "Local")
            nc.sync.dma_start(st_in[:], stats_g[g][:])
            nc.gpsimd.collective_compute(
                "AllGather", ALU.bypass,
                replica_groups=[list(range(n_cores))],
                ins=[st_in[:].opt()], outs=[st_out[:].opt()],
            )
            gath = stat.tile([P, 8, G * 4], F32, tag=f"gath{g}",
                             name=f"gath{g}")
            nc.sync.dma_start(gath[:], st_out[:].rearrange("c p f -> p c f"))
            return gath

        def merge_and_coefs(g, gath):
            """Cross-core moment merge + coefs, batched over both mc chunks.

            Runs on DVE (tile serializes the Pool queue through pending
            collectives' completions, so Pool can't help mid-stream)."""
            members = GROUPS[g]
            G = len(members)
            F = G * 4
            t4 = tmp.tile([P, 4, F], F32, tag=f"t4_{g}", name=f"t4_{g}")
            nc.vector.tensor_tensor(t4[:], gath[:, 0:4], gath[:, 4:8],
                                    op=ALU.add)
            t2 = tmp.tile([P, 2, F], F32, tag=f"t2_{g}", name=f"t2_{g}")
            nc.vector.tensor_tensor(t2[:], t4[:, 0:2], t4[:, 2:4], op=ALU.add)
            t1 = tmp.tile([P, F], F32, tag=f"t1_{g}", name=f"t1_{g}")
            nc.vector.tensor_tensor(t1[:], t2[:, 0], t2[:, 1], op=ALU.add)
            # Sum_c mean_c^2: square the gathered means (strided lanes) + tree
            sq = tmp.tile([P, 8, G, 2], F32, tag=f"sq_{g}", name=f"sq_{g}")
            gmean = gath[:].rearrange("p c (g m f) -> p c g m f",
                                      m=2, f=2)[:, :, :, :, 0]
            nc.vector.tensor_tensor(sq[:], gmean, gmean, op=ALU.mult)
            q4 = tmp.tile([P, 4, G, 2], F32, tag=f"q4_{g}", name=f"q4_{g}")
            nc.vector.tensor_tensor(q4[:], sq[:, 0:4], sq[:, 4:8], op=ALU.add)
            q2 = tmp.tile([P, 2, G, 2], F32, tag=f"q2_{g}", name=f"q2_{g}")
            nc.vector.tensor_tensor(q2[:], q4[:, 0:2], q4[:, 2:4], op=ALU.add)
            q1 = tmp.tile([P, G, 2], F32, tag=f"q1_{g}", name=f"q1_{g}")
            nc.vector.tensor_tensor(q1[:], q2[:, 0], q2[:, 1], op=ALU.add)

            # [P, G, 2mc] views over both mc at once
            t1v = t1[:].rearrange("p (g m f) -> p g m f", m=2, f=2)
            means = t1v[:, :, :, 0]
            vars_ = t1v[:, :, :, 1]
            G2 = G * 2
            a_t = stat.tile([P, G, 2], F32, tag=f"a{g}", name=f"a{g}")
            lo_t = stat.tile([P, G, 2], F32, tag=f"lo{g}", name=f"lo{g}")
            hi_t = stat.tile([P, G, 2], F32, tag=f"hi{g}", name=f"hi{g}")
            mu = tmp.tile([P, G, 2], F32, tag=f"mu{g}", name=f"mu{g}")
            nc.vector.tensor_scalar(mu[:], means, 1.0 / n_cores, None,
                                    op0=ALU.mult)
            var = tmp.tile([P, G, 2], F32, tag=f"var{g}", name=f"var{g}")
            nc.vector.tensor_tensor(var[:], vars_, q1[:], op=ALU.add)
            nc.vector.tensor_scalar(var[:], var[:], 1.0 / n_cores, None,
                                    op0=ALU.mult)
            musq = tmp.tile([P, G, 2], F32, tag=f"musq{g}", name=f"musq{g}")
            nc.vector.tensor_tensor(musq[:], mu[:], mu[:], op=ALU.mult)
            nc.vector.tensor_tensor(var[:], var[:], musq[:], op=ALU.subtract)
            nc.vector.tensor_scalar(var[:], var[:], EPS, None, op0=ALU.add)
            std = tmp.tile([P, G, 2], F32, tag=f"std{g}", name=f"std{g}")
            nc.scalar.activation(std[:], var[:], AF.Sqrt)
            inv = tmp.tile([P, G, 2], F32, tag=f"inv{g}", name=f"inv{g}")
            nc.vector.reciprocal(inv[:], std[:])
            # gamma/beta views [P, G, 2mc]: coef layout [P, mc, 2j + (0|1)];
            # group members are evenly spaced in j, so one strided AP covers
            # all (j, mc) pairs.
            j0 = members[0]
            jstep = (members[1] - members[0]) if G > 1 else 1
            # build [P, G, 2] strided views by AP arithmetic
            gam = coef_sb[:].rearrange("p m f -> p f m")[
                :, 2 * j0:2 * j0 + (G - 1) * 2 * jstep + 1:2 * jstep, :]
            bet = coef_sb[:].rearrange("p m f -> p f m")[
                :, 2 * j0 + 1:2 * j0 + 1 + (G - 1) * 2 * jstep + 1:2 * jstep, :]
            d_ = tmp.tile([P, G, 2], F32, tag=f"d{g}", name=f"d{g}")
            nc.vector.tensor_tensor(a_t[:], gam, inv[:], op=ALU.mult)
            nc.vector.tensor_tensor(d_[:], mu[:], a_t[:], op=ALU.mult)
            nc.vector.tensor_tensor(d_[:], bet, d_[:], op=ALU.subtract)
            inva = tmp.tile([P, G, 2], F32, tag=f"inva{g}", name=f"inva{g}")
            nc.vector.reciprocal(inva[:], a_t[:])
            nc.vector.tensor_scalar(lo_t[:], d_[:], -1.0, -1.0,
                                    op0=ALU.mult, op1=ALU.add)
            nc.vector.tensor_tensor(lo_t[:], lo_t[:], inva[:], op=ALU.mult)
            nc.vector.tensor_scalar(hi_t[:], d_[:], -1.0, 1.0,
                                    op0=ALU.mult, op1=ALU.add)
            nc.vector.tensor_tensor(hi_t[:], hi_t[:], inva[:], op=ALU.mult)
            # s0 += sum_j d_j per mc (reduce over the G axis, mc innermost)
            dsum = tmp.tile([P, 2], F32, tag=f"dsum{g}", name=f"dsum{g}")
            if G > 1:
                nc.vector.reduce_sum(dsum[:], d_[:].rearrange("p g m -> p m g"),
                                     axis=AX.X)
            else:
                nc.vector.tensor_copy(dsum[:], d_[:, 0])
            nc.vector.tensor_tensor(s0[:], s0[:], dsum[:], op=ALU.add)
            if g == 0:
                ay = tmp.tile([P, 2], F32, tag="ay")
                ypb = tmp.tile([P, 2], F32, tag="ypb")
                for mc in range(2):
                    nc.vector.tensor_copy(ypb[:, mc:mc + 1], ypool[mc][:])
                nc.vector.tensor_tensor(ay[:], a_t[:, 0], ypb[:], op=ALU.mult)
                nc.vector.tensor_tensor(s0[:], s0[:], ay[:], op=ALU.add)
            coefs[g] = dict(a=a_t, lo=lo_t, hi=hi_t)

        def apply_branch(g, gi, j):
            """In-place on y16: u = a*clip(y, lo, hi), two DVE fp16 passes
            (4x mode).  The branch sums are deferred to tail add-trees."""
            c = coefs[g]
            for mc in range(2):
                yt = y16_all[j][mc]
                nc.vector.tensor_scalar(yt[:], yt[:],
                                        c["lo"][:, gi, mc:mc + 1],
                                        c["hi"][:, gi, mc:mc + 1],
                                        op0=ALU.max, op1=ALU.min)
                nc.vector.tensor_scalar(yt[:], yt[:], c["a"][:, gi, mc:mc + 1],
                                        None, op0=ALU.mult)

        # group bookkeeping: branch j -> (g, gi); stats slot for conv branches
        slot = {}
        for g, members in enumerate(GROUPS):
            for gi, j in enumerate(members):
                slot[j] = (g, gi)

        gathers = {}
        pending = []          # [(emit_after_branch, fn)]

        def emit_tile(j, tap0, r, mc, h0, nr, yt):
            taps = ([(tap0, 1, 1)] if r is None else
                    [(tap0 + 3 * ky + kx, ky, kx)
                     for ky in range(3) for kx in range(3)])
            n = nr * PW
            acc = ppool.tile([P, 5 * PW], F32, tag="acc")
            for i_mm, (tap, ky, kx) in enumerate(taps):
                rr = 0 if r is None else r
                pos = (GUARD + (PAD + h0 + rr * (ky - 1)) * PW
                       + rr * (kx - 1))
                rhs = sxp[:, :, pos:pos + n]
                nc.tensor.matmul(acc[:, 0:n], wdr(tap * 2 + mc), rhs,
                                 start=(i_mm == 0),
                                 stop=(i_mm == len(taps) - 1),
                                 perf_mode=DR)
            acc3 = acc[:, 0:n].rearrange("p (r c) -> p r c", c=PW)
            useful = acc3[:, :, PAD:PAD + W]
            nc.scalar.activation(yt[:, h0 * W:(h0 + nr) * W], useful, AF.Copy)

        def emit_stats(j, mc, yt):
            # BN stats: 8 chunks of 512 + aggregate -> (mean, var)
            g, gi = slot[j]
            bn6 = tmp.tile([P, NCHUNK, 6], F32, tag="bn6")
            y3 = yt[:].rearrange("p (c e) -> p c e", e=512)
            for cch in range(NCHUNK):
                nc.vector.bn_stats(bn6[:, cch], y3[:, cch])
            nc.vector.bn_aggr(stats_g[g][:, gi, mc, 0:2], bn6[:])

        def emit_section(defs):
            """Emit one or more branches with their tile jobs interleaved."""
            jobs = []   # per branch: list of (j, tap0, r, mc, h0, nr, yt, last)
            for (j, tap0, r) in defs:
                y16 = {mc: ybuf.tile([P, HW], F16, tag=f"y{j}_{mc}",
                                     name=f"y{j}_{mc}") for mc in range(2)}
                y16_all[j] = y16
                lst = []
                for mc in range(2):
                    for it, (h0, nr) in enumerate(ROWTILES):
                        lst.append((j, tap0, r, mc, h0, nr, y16[mc],
                                    it == NRT - 1))
                jobs.append(lst)
            # round-robin across branches, one tile job at a time
            iters = [iter(l) for l in jobs]
            live = list(range(len(iters)))
            while live:
                for bi_ in list(live):
                    try:
                        (j, tap0, r, mc, h0, nr, yt, last) = next(iters[bi_])
                    except StopIteration:
                        live.remove(bi_)
                        continue
                    emit_tile(j, tap0, r, mc, h0, nr, yt)
                    if last:
                        emit_stats(j, mc, yt)

        def emit_group_apply(gg):
            merge_and_coefs(gg, gathers[gg])
            for gi2, jj in enumerate(GROUPS[gg]):
                if jj != 0:
                    apply_branch(gg, gi2, jj)

        for bi, defs in enumerate(SECTIONS):
            emit_section(defs)
            if bi == 0:
                emit_pool_branch()
            for (j, tap0, r) in defs:
                g, gi = slot[j]
                # last member of group g? -> issue its AllGather now
                if gi == len(GROUPS[g]) - 1:
                    gathers[g] = issue_gather(g)
                    pending.append((bi + 1, g))
            # emit merges/applies whose delay has elapsed (gathers above went
            # first so a pending AllGather is never queued behind mads)
            for (when, gg) in [p for p in pending]:
                if when == bi:
                    pending.remove((when, gg))
                    emit_group_apply(gg)

        # partial add-tree over the already-scaled non-tail branches:
        # q = (r1+r4) + (1x1+r8), in place — executes inside the last
        # gather's window on DVE (tile's Pool queue is blocked until that
        # gather completes)
        ptree = {}
        for mc in range(2):
            pa = y16_all[2][mc]                      # r1 (in place)
            nc.vector.tensor_tensor(pa[:], pa[:], y16_all[3][mc][:],
                                    op=ALU.add)
            pb = y16_all[1][mc]                      # 1x1 (in place)
            nc.vector.tensor_tensor(pb[:], pb[:], y16_all[4][mc][:],
                                    op=ALU.add)
            nc.vector.tensor_tensor(pa[:], pa[:], pb[:], op=ALU.add)
            ptree[mc] = pa

        # drain the tail group (r12): coefs + clip/scale (mc1 first so the
        # Pool-side mc1 chain below starts as early as possible)
        for (when, gg) in pending:
            merge_and_coefs(gg, gathers[gg])
            c = coefs[gg]
            for mc in (1, 0):
                yt = y16_all[5][mc]
                nc.vector.tensor_scalar(yt[:], yt[:], c["lo"][:, 0, mc:mc + 1],
                                        c["hi"][:, 0, mc:mc + 1],
                                        op0=ALU.max, op1=ALU.min)
                nc.vector.tensor_scalar(yt[:], yt[:], c["a"][:, 0, mc:mc + 1],
                                        None, op0=ALU.mult)

        # ---- final: s = q + r12 + s0, store as fp16.  DVE takes mc0, the
        # (post-gather free) Pool takes mc1.
        for mc in (1, 0):
            eng = nc.gpsimd if mc == 1 else nc.vector
            pa = ptree[mc]
            eng.tensor_tensor(pa[:], pa[:], y16_all[5][mc][:], op=ALU.add)
            for t in range(2):
                sf = sbout.tile([P, 2048], F16, tag="sf")
                eng.tensor_scalar(sf[:], pa[:, t * 2048:(t + 1) * 2048],
                                  1.0, s0[:, mc:mc + 1],
                                  op0=ALU.mult, op1=ALU.add)
                deng = nc.sync if t == 0 else nc.scalar
                deng.dma_start(
                    out.ap()[mc * P:(mc + 1) * P].rearrange(
                        "m h w -> m (h w)")[:, t * 2048:(t + 1) * 2048],
                    sf[:])

    nc.compile()
    return nc


def pack_weights(w_pool, w1, w3):
    """Host filter transform: sign -> DoubleRow k-interleave, fp8.

    wt[k, t*2+mc, i, m] = sign(W_t[mc*128+m, i*128+k]); block (t*2+mc) is the
    stationary [2, 128] operand for logical tap t / out-channel chunk mc.
    """
    mats = [np.sign(np.asarray(w_pool, np.float32).reshape(COUT, CIN)),
            np.sign(np.asarray(w1, np.float32).reshape(COUT, CIN))]
    w3 = np.asarray(w3, np.float32)
    for i in range(len(RATES)):
        for ky in range(3):
            for kx in range(3):
                mats.append(np.sign(w3[i, :, :, ky, kx]))
    wt = np.zeros((P, NT * 2, 2, P), np.float32)  # [k, blk, i, m]
    for t, m in enumerate(mats):
        for mc in range(2):
            for i in range(2):
                blk = m[mc * P:(mc + 1) * P, i * P:(i + 1) * P]   # [m, k]
                wt[:, t * 2 + mc, i, :] = blk.T
    return wt.astype(mybir.dt.np(FP8))


def pack_coef(g_pool, b_pool, g1, b1, g3, b3):
    gs = [g_pool, g1] + [g3[i] for i in range(len(RATES))]
    bs = [b_pool, b1] + [b3[i] for i in range(len(RATES))]
    coef = np.zeros((P, 2, 12), np.float32)
    for j in range(6):
        g = np.asarray(gs[j], np.float32)
        b = np.asarray(bs[j], np.float32)
        for mc in range(2):
            coef[:, mc, 2 * j] = g[mc * P:(mc + 1) * P]
            coef[:, mc, 2 * j + 1] = b[mc * P:(mc + 1) * P]
    return coef


_NC = None


def _get_nc():
    global _NC
    if _NC is None:
        _NC = build(N_CORES)
    return _NC


def make_in_maps(x, w_pool, g_pool, b_pool, w1, g1, b1, w3, g3, b3):
    x = np.asarray(x, np.float32)
    wt = pack_weights(w_pool, w1, w3)
    coef = pack_coef(g_pool, b_pool, g1, b1, g3, b3)
    return [
        {"xs": np.ascontiguousarray(x[c]), "wt": wt, "coef": coef}
        for c in range(x.shape[0])
    ]


def kernel(x, w_pool, g_pool, b_pool, w1, g1, b1, w3, g3, b3):
    nc = _get_nc()
    in_maps = make_in_maps(x, w_pool, g_pool, b_pool, w1, g1, b1, w3, g3, b3)
    res = run_bass_kernel_spmd(nc, in_maps, core_ids=list(range(N_CORES)))
    return np.stack([res.results[c]["out"] for c in range(N_CORES)],
                    axis=0).astype(np.float32)


# revision 6
# speedup vs baseline: 3.0660x; 1.1125x over previous
"""BinASPP Trainium2 kernel (Bass/Tile), SPMD over 8 NeuronCores.

Strategy (v2)
-------------
Data-parallel over batch: N=8 images -> 1 image per core.  binarize() forward
== sign(), so every conv is a matmul over {-1,+1} values: exact in fp8e4 with
integer accumulation in fp32 PSUM.  A dilated 3x3 conv is 9 shifted 1x1 convs
(taps) over a zero-padded sign image resident in SBUF; a DoubleRow matmul
contracts all K=256 input channels at once, streaming contiguous runs of 5
padded rows (440 cols) per PSUM bank (~0.21 ns/output column).

Engine split (the v1 bottlenecks were ACT at ~100% busy — accum_out aux reads
and a Square pass per tile — and 6 serialized 28us AllReduces):
 - ACT: plain PSUM->SBUF fp16 copies of the useful columns only.  y fp16 is
   exact (even integers, |y| <= 2304).
 - DVE: per-branch BN stats via bn_stats (8x512 chunks) + bn_aggr ->
   (mean, var); after each group's gather, clip+scale u_j = a_j*clip(y_j)
   in place on y16 (fp16 tensor_scalar runs in 4x DVE mode).
 - Branch summation is deferred: partial add-trees q = sum(u_j) run inside
   the last gather's latency window; the final s = q + u_r12 + s0 splits
   mc0 on DVE / mc1 on Pool.  (Tile serializes every engine queue through a
   pending collective's COMPLETION, so Pool is unusable until the last
   gather lands — nothing stream-critical may be queued behind a
   collective.)
 - Sync-BN: per-branch-group AllGather (15.4us, vs AllReduce's 28.1us) of
   (mean, var) pairs; each core merges the 8 cores' moments locally
   (var_g = E[var] + E[mean^2] - E[mean]^2, coef math batched over both mc
   chunks).  Groups {pool,r1} {r4,1x1} {1x1? -> r8} {r12} pipeline under the
   matmul stream so only r12's gather sits in the tail.
 - The 1x1 branch's 26 tiles are interleaved into the r4 section so its
   PSUM drains never burst the ACT queue; x loads are 16-row blocks
   alternating sync/scalar DMA queues with the lhsT weight load split
   around them (global DMA is a single serialized resource).
BN offsets d_j (+ the pool branch's a*y_pool + d) fold into a per-channel
s0 added in the final pass; output is stored fp16 and widened on host.
"""

import numpy as np
import ml_dtypes
from contextlib import ExitStack

import concourse.bass as bass
import concourse.bacc as bacc
import concourse.mybir as mybir
import concourse.tile as tile
from concourse.bass_utils import run_bass_kernel_spmd

AF = mybir.ActivationFunctionType
ALU = mybir.AluOpType
AX = mybir.AxisListType
F32 = mybir.dt.float32
F16 = mybir.dt.float16
FP8 = mybir.dt.float8e4
DR = mybir.MatmulPerfMode.DoubleRow

P = 128
CIN = 256
COUT = 256
H = W = 64
HW = H * W
PAD = 12                      # max dilation rate
PH = PW = H + 2 * PAD         # 88
GUARD = 16                    # fp8 guard elements before/after each image
ILEN = GUARD + PH * PW + GUARD  # 7776 (multiple of 16 -> DR stride rule)
RATES = (1, 4, 8, 12)
NT = 2 + 9 * len(RATES)       # 38 tap matrices: pool, 1x1, 4 branches x 9
EPS = 1e-5
N_CORES = 8
# spatial tiles: runs of full padded rows, 5 rows (440 cols) per PSUM bank
ROWTILES = [(5 * t, 5) for t in range(12)] + [(60, 4)]
NRT = len(ROWTILES)           # 13
NCHUNK = 8                    # bn_stats chunks of 512 over HW=4096
# branch ids: 0=pool, 1=1x1, 2=r1, 3=r4, 4=r8, 5=r12 (coef layout order).
# Section emission order: r1 first (matmuls start as soon as its rows are
# signed); the 1x1's 26 tiles are interleaved INTO the r4 section so its
# ~12us of PSUM drains never burst the ACT queue; r12 last so its gather is
# the only tail collective.  Each inner list is one emission unit.
SECTIONS = [[(2, 2, 1)], [(3, 11, 4), (1, 1, None)], [(4, 20, 8)],
            [(5, 29, 12)]]
# branch groups for the stats AllGathers (stats-completion order)
GROUPS = [[0, 2], [3], [1, 4], [5]]


def build(n_cores: int = N_CORES):
    nc = bacc.Bacc(
        "TRN2",
        target_bir_lowering=False,
        debug=False,
        enable_asserts=False,
        num_devices=n_cores,
    )
    xs = nc.dram_tensor("xs", [CIN, H, W], F32, kind="ExternalInput")
    wt = nc.dram_tensor("wt", [P, NT * 2, 2, P], FP8, kind="ExternalInput")
    coef = nc.dram_tensor("coef", [P, 2, 12], F32, kind="ExternalInput")
    out = nc.dram_tensor("out", [COUT, H, W], F16, kind="ExternalOutput")

    with tile.TileContext(nc) as tc, ExitStack() as ctx:
        const = ctx.enter_context(tc.tile_pool(name="const", bufs=1))
        xload = ctx.enter_context(tc.tile_pool(name="xload", bufs=2))
        ppool = ctx.enter_context(
            tc.tile_pool(name="ppool", bufs=6, space=bass.MemorySpace.PSUM))
        psmall = ctx.enter_context(
            tc.tile_pool(name="psmall", bufs=2, space=bass.MemorySpace.PSUM))
        ybuf = ctx.enter_context(tc.tile_pool(name="ybuf", bufs=1))
        stat = ctx.enter_context(tc.tile_pool(name="stat", bufs=1))
        tmp = ctx.enter_context(tc.tile_pool(name="tmp", bufs=4))
        sbout = ctx.enter_context(tc.tile_pool(name="sbout", bufs=2))
        dram = ctx.enter_context(
            tc.tile_pool(name="dram", bufs=1, space=bass.MemorySpace.DRAM))

        # ---- weights + bn coefficient load.  DMA transfers serialize on the
        # global DMA engines, so order by need: taps for pool/1x1/r1 first,
        # then the x image chunks, then the remaining taps (r4/r8/r12, not
        # needed until ~40us in).
        NBLK1 = 22            # blocks 0..21: pool, 1x1, r1 taps
        lhsT = const.tile([P, NT * 2, 2, P], FP8, tag="lhsT")
        nc.sync.dma_start(lhsT[:, 0:NBLK1], wt.ap()[:, 0:NBLK1])
        coef_sb = const.tile([P, 2, 12], F32, tag="coef")
        nc.scalar.dma_start(coef_sb[:], coef.ap())

        def wdr(blk):
            return lhsT[:, blk]          # [P, 2, P] fp8, k-interleaved

        # ---- x -> padded k-interleaved sign image (fp8) + pooled sign
        sxp = const.tile([P, 2, ILEN], FP8, tag="sxp")
        for i in range(2):
            # zero only pad/guard areas (interior is overwritten by Sign)
            nc.gpsimd.memset(sxp[:, i, 0:GUARD + PAD * PW], 0.0)
            nc.gpsimd.memset(sxp[:, i, GUARD + (PAD + H) * PW:ILEN], 0.0)
            off0 = GUARD + PAD * PW - PAD
            seams = sxp[:, i, off0:off0 + (H + 1) * PW].rearrange(
                "p (r c) -> p r c", c=PW)[:, :, 0:2 * PAD]
            nc.gpsimd.memset(seams, 0.0)
        spool = const.tile([P, 2, 16], FP8, tag="spool")  # 16-wide: DR step rule
        XBLK = 4
        XR = H // XBLK            # 16 rows per block
        interiors = [
            sxp[:, kc, GUARD:GUARD + PH * PW].rearrange(
                "p (r c) -> p r c", c=PW)[:, PAD:PAD + H, PAD:PAD + W]
            for kc in range(2)]
        xs4 = {kc: xload.tile([P, XBLK], F32, tag=f"xs4_{kc}",
                              name=f"xs4_{kc}") for kc in range(2)}
        # interleave kc within each row-block so early rows of BOTH k-chunks
        # are signed first (matmuls contract both chunks)
        for b in range(XBLK):
            for kc in range(2):
                xsb = xload.tile([P, XR, W], F32, tag="xsb")
                eng = nc.sync if kc == 0 else nc.scalar
                eng.dma_start(
                    xsb[:], xs.ap()[kc * P:(kc + 1) * P, b * XR:(b + 1) * XR])
                nc.vector.reduce_sum(xs4[kc][:, b:b + 1], xsb[:], axis=AX.XY)
                nc.scalar.activation(
                    interiors[kc][:, b * XR:(b + 1) * XR], xsb[:], AF.Sign)
        for kc in range(2):
            xsum = xload.tile([P, 1], F32, tag=f"xsum_{kc}",
                              name=f"xsum_{kc}")
            nc.vector.reduce_sum(xsum[:], xs4[kc][:], axis=AX.X)
            nc.scalar.activation(spool[:, kc, 0:1], xsum[:], AF.Sign)
        # remaining taps (r4/r8/r12) after the x image is on its way
        nc.sync.dma_start(lhsT[:, NBLK1:], wt.ap()[:, NBLK1:])

        # per-branch-group stats tiles: [P, G, 2mc, (mean|var)]
        stats_g = {}
        for g, members in enumerate(GROUPS):
            stats_g[g] = stat.tile([P, len(members), 2, 2], F32,
                                   tag=f"stats_g{g}", name=f"stats_g{g}")
        # deferred per-channel offset (sum of d_j + pool-branch value)
        s0 = stat.tile([P, 2], F32, tag="s0", name="s0")
        nc.vector.memset(s0[:], 0.0)

        # ---- pool branch (j=0): y_pool = sign(W_pool) @ sign(mean(x)).
        # Emitted AFTER the first conv section so its wait on spool (ready
        # only once the whole image is loaded) doesn't head-of-line block the
        # PE queue in front of the r1 matmuls.
        ypool = {}

        def emit_pool_branch():
            for mc in range(2):
                yp = psmall.tile([P, 1], F32, tag="yp")
                nc.tensor.matmul(yp[:], wdr(0 * 2 + mc), spool[:, :, 0:1],
                                 start=True, stop=True, perf_mode=DR)
                ys = stat.tile([P, 1], F32, tag=f"ypool{mc}",
                               name=f"ypool{mc}")
                nc.scalar.activation(ys[:], yp[:], AF.Copy)
                ypool[mc] = ys
                # stats: mean = y (count 1 per core), var = 0
                nc.vector.tensor_copy(stats_g[0][:, 0, mc, 0:1], ys[:])
                nc.vector.memset(stats_g[0][:, 0, mc, 1:2], 0.0)

        # ---- group coef state (filled by issue/merge closures)
        coefs = {}   # g -> dict(a=, lo=, hi=) tiles [P, 2, G]
        y16_all = {}  # branch j -> {mc: y16 tile}

        def issue_gather(g):
            members = GROUPS[g]
            G = len(members)
            st_in = dram.tile([P, G * 4], F32, tag=f"st_in{g}")
            st_out = dram.tile([8, P, G * 4], F32, tag=f"st_out{g}",
                               addr_space="Shared" if n_cores > 4 else "Local")
            nc.sync.dma_start(st_in[:], stats_g[g][:])
            nc.gpsimd.collective_compute(
                "AllGather", ALU.bypass,
                replica_groups=[list(range(n_cores))],
                ins=[st_in[:].opt()], outs=[st_out[:].opt()],
            )
            gath = stat.tile([P, 8, G * 4], F32, tag=f"gath{g}",
                             name=f"gath{g}")
            nc.sync.dma_start(gath[:], st_out[:].rearrange("c p f -> p c f"))
            return gath

        def merge_and_coefs(g, gath):
            """Cross-core moment merge + coefs, batched over both mc chunks.

            Runs on DVE (tile serializes the Pool queue through pending
            collectives' completions, so Pool can't help mid-stream)."""
            members = GROUPS[g]
            G = len(members)
            F = G * 4
            t4 = tmp.tile([P, 4, F], F32, tag=f"t4_{g}", name=f"t4_{g}")
            nc.vector.tensor_tensor(t4[:], gath[:, 0:4], gath[:, 4:8],
                                    op=ALU.add)
            t2 = tmp.tile([P, 2, F], F32, tag=f"t2_{g}", name=f"t2_{g}")
            nc.vector.tensor_tensor(t2[:], t4[:, 0:2], t4[:, 2:4], op=ALU.add)
            t1 = tmp.tile([P, F], F32, tag=f"t1_{g}", name=f"t1_{g}")
            nc.vector.tensor_tensor(t1[:], t2[:, 0], t2[:, 1], op=ALU.add)
            # Sum_c mean_c^2: square the gathered means (strided lanes) + tree
            sq = tmp.tile([P, 8, G, 2], F32, tag=f"sq_{g}", name=f"sq_{g}")
            gmean = gath[:].rearrange("p c (g m f) -> p c g m f",
                                      m=2, f=2)[:, :, :, :, 0]
            nc.vector.tensor_tensor(sq[:], gmean, gmean, op=ALU.mult)
            q4 = tmp.tile([P, 4, G, 2], F32, tag=f"q4_{g}", name=f"q4_{g}")
            nc.vector.tensor_tensor(q4[:], sq[:, 0:4], sq[:, 4:8], op=ALU.add)
            q2 = tmp.tile([P, 2, G, 2], F32, tag=f"q2_{g}", name=f"q2_{g}")
            nc.vector.tensor_tensor(q2[:], q4[:, 0:2], q4[:, 2:4], op=ALU.add)
            q1 = tmp.tile([P, G, 2], F32, tag=f"q1_{g}", name=f"q1_{g}")
            nc.vector.tensor_tensor(q1[:], q2[:, 0], q2[:, 1], op=ALU.add)

            # [P, G, 2mc] views over both mc at once
            t1v = t1[:].rearrange("p (g m f) -> p g m f", m=2, f=2)
            means = t1v[:, :, :, 0]
            vars_ = t1v[:, :, :, 1]
            G2 = G * 2
            a_t = stat.tile([P, G, 2], F32, tag=f"a{g}", name=f"a{g}")
            lo_t = stat.tile([P, G, 2], F32, tag=f"lo{g}", name=f"lo{g}")
            hi_t = stat.tile([P, G, 2], F32, tag=f"hi{g}", name=f"hi{g}")
            mu = tmp.tile([P, G, 2], F32, tag=f"mu{g}", name=f"mu{g}")
            nc.vector.tensor_scalar(mu[:], means, 1.0 / n_cores, None,
                                    op0=ALU.mult)
            var = tmp.tile([P, G, 2], F32, tag=f"var{g}", name=f"var{g}")
            nc.vector.tensor_tensor(var[:], vars_, q1[:], op=ALU.add)
            nc.vector.tensor_scalar(var[:], var[:], 1.0 / n_cores, None,
                                    op0=ALU.mult)
            musq = tmp.tile([P, G, 2], F32, tag=f"musq{g}", name=f"musq{g}")
            nc.vector.tensor_tensor(musq[:], mu[:], mu[:], op=ALU.mult)
            nc.vector.tensor_tensor(var[:], var[:], musq[:], op=ALU.subtract)
            nc.vector.tensor_scalar(var[:], var[:], EPS, None, op0=ALU.add)
            std = tmp.tile([P, G, 2], F32, tag=f"std{g}", name=f"std{g}")
            nc.scalar.activation(std[:], var[:], AF.Sqrt)
            inv = tmp.tile([P, G, 2], F32, tag=f"inv{g}", name=f"inv{g}")
            nc.vector.reciprocal(inv[:], std[:])
            # gamma/beta views [P, G, 2mc]: coef layout [P, mc, 2j + (0|1)];
            # group members are evenly spaced in j, so one strided AP covers
            # all (j, mc) pairs.
            j0 = members[0]
            jstep = (members[1] - members[0]) if G > 1 else 1
            # build [P, G, 2] strided views by AP arithmetic
            gam = coef_sb[:].rearrange("p m f -> p f m")[
                :, 2 * j0:2 * j0 + (G - 1) * 2 * jstep + 1:2 * jstep, :]
            bet = coef_sb[:].rearrange("p m f -> p f m")[
                :, 2 * j0 + 1:2 * j0 + 1 + (G - 1) * 2 * jstep + 1:2 * jstep, :]
            d_ = tmp.tile([P, G, 2], F32, tag=f"d{g}", name=f"d{g}")
            nc.vector.tensor_tensor(a_t[:], gam, inv[:], op=ALU.mult)
            nc.vector.tensor_tensor(d_[:], mu[:], a_t[:], op=ALU.mult)
            nc.vector.tensor_tensor(d_[:], bet, d_[:], op=ALU.subtract)
            inva = tmp.tile([P, G, 2], F32, tag=f"inva{g}", name=f"inva{g}")
            nc.vector.reciprocal(inva[:], a_t[:])
            nc.vector.tensor_scalar(lo_t[:], d_[:], -1.0, -1.0,
                                    op0=ALU.mult, op1=ALU.add)
            nc.vector.tensor_tensor(lo_t[:], lo_t[:], inva[:], op=ALU.mult)
            nc.vector.tensor_scalar(hi_t[:], d_[:], -1.0, 1.0,
                                    op0=ALU.mult, op1=ALU.add)
            nc.vector.tensor_tensor(hi_t[:], hi_t[:], inva[:], op=ALU.mult)
            # s0 += sum_j d_j per mc (reduce over the G axis, mc innermost)
            dsum = tmp.tile([P, 2], F32, tag=f"dsum{g}", name=f"dsum{g}")
            if G > 1:
                nc.vector.reduce_sum(dsum[:], d_[:].rearrange("p g m -> p m g"),
                                     axis=AX.X)
            else:
                nc.vector.tensor_copy(dsum[:], d_[:, 0])
            nc.vector.tensor_tensor(s0[:], s0[:], dsum[:], op=ALU.add)
            if g == 0:
                ay = tmp.tile([P, 2], F32, tag="ay")
                ypb = tmp.tile([P, 2], F32, tag="ypb")
                for mc in range(2):
                    nc.vector.tensor_copy(ypb[:, mc:mc + 1], ypool[mc][:])
                nc.vector.tensor_tensor(ay[:], a_t[:, 0], ypb[:], op=ALU.mult)
                nc.vector.tensor_tensor(s0[:], s0[:], ay[:], op=ALU.add)
            coefs[g] = dict(a=a_t, lo=lo_t, hi=hi_t)

        def apply_branch(g, gi, j):
            """In-place on y16: u = a*clip(y, lo, hi), two DVE fp16 passes
            (4x mode).  The branch sums are deferred to tail add-trees."""
            c = coefs[g]
            for mc in range(2):
                yt = y16_all[j][mc]
                nc.vector.tensor_scalar(yt[:], yt[:],
                                        c["lo"][:, gi, mc:mc + 1],
                                        c["hi"][:, gi, mc:mc + 1],
                                        op0=ALU.max, op1=ALU.min)
                nc.vector.tensor_scalar(yt[:], yt[:], c["a"][:, gi, mc:mc + 1],
                                        None, op0=ALU.mult)

        # group bookkeeping: branch j -> (g, gi); stats slot for conv branches
        slot = {}
        for g, members in enumerate(GROUPS):
            for gi, j in enumerate(members):
                slot[j] = (g, gi)

        gathers = {}
        pending = []          # [(emit_after_branch, fn)]

        def emit_tile(j, tap0, r, mc, h0, nr, yt):
            taps = ([(tap0, 1, 1)] if r is None else
                    [(tap0 + 3 * ky + kx, ky, kx)
                     for ky in range(3) for kx in range(3)])
            n = nr * PW
            acc = ppool.tile([P, 5 * PW], F32, tag="acc")
            for i_mm, (tap, ky, kx) in enumerate(taps):
                rr = 0 if r is None else r
                pos = (GUARD + (PAD + h0 + rr * (ky - 1)) * PW
                       + rr * (kx - 1))
                rhs = sxp[:, :, pos:pos + n]
                nc.tensor.matmul(acc[:, 0:n], wdr(tap * 2 + mc), rhs,
                                 start=(i_mm == 0),
                                 stop=(i_mm == len(taps) - 1),
                                 perf_mode=DR)
            acc3 = acc[:, 0:n].rearrange("p (r c) -> p r c", c=PW)
            useful = acc3[:, :, PAD:PAD + W]
            nc.scalar.activation(yt[:, h0 * W:(h0 + nr) * W], useful, AF.Copy)

        def emit_stats(j, mc, yt):
            # BN stats: 8 chunks of 512 + aggregate -> (mean, var)
            g, gi = slot[j]
            bn6 = tmp.tile([P, NCHUNK, 6], F32, tag="bn6")
            y3 = yt[:].rearrange("p (c e) -> p c e", e=512)
            for cch in range(NCHUNK):
                nc.vector.bn_stats(bn6[:, cch], y3[:, cch])
            nc.vector.bn_aggr(stats_g[g][:, gi, mc, 0:2], bn6[:])

        def emit_section(defs):
            """Emit one or more branches with their tile jobs interleaved."""
            jobs = []   # per branch: list of (j, tap0, r, mc, h0, nr, yt, last)
            for (j, tap0, r) in defs:
                y16 = {mc: ybuf.tile([P, HW], F16, tag=f"y{j}_{mc}",
                                     name=f"y{j}_{mc}") for mc in range(2)}
                y16_all[j] = y16
                lst = []
                for mc in range(2):
                    for it, (h0, nr) in enumerate(ROWTILES):
                        lst.append((j, tap0, r, mc, h0, nr, y16[mc],
                                    it == NRT - 1))
                jobs.append(lst)
            # round-robin across branches, one tile job at a time
            iters = [iter(l) for l in jobs]
            live = list(range(len(iters)))
            while live:
                for bi_ in list(live):
                    try:
                        (j, tap0, r, mc, h0, nr, yt, last) = next(iters[bi_])
                    except StopIteration:
                        live.remove(bi_)
                        continue
                    emit_tile(j, tap0, r, mc, h0, nr, yt)
                    if last:
                        emit_stats(j, mc, yt)

        def emit_group_apply(gg):
            merge_and_coefs(gg, gathers[gg])
            for gi2, jj in enumerate(GROUPS[gg]):
                if jj != 0:
                    apply_branch(gg, gi2, jj)

        for bi, defs in enumerate(SECTIONS):
            emit_section(defs)
            if bi == 0:
                emit_pool_branch()
            for (j, tap0, r) in defs:
                g, gi = slot[j]
                # last member of group g? -> issue its AllGather now
                if gi == len(GROUPS[g]) - 1:
                    gathers[g] = issue_gather(g)
                    pending.append((bi + 1, g))
            # emit merges/applies whose delay has elapsed (gathers above went
            # first so a pending AllGather is never queued behind mads)
            for (when, gg) in [p for p in pending]:
                if when == bi:
                    pending.remove((when, gg))
                    emit_group_apply(gg)

        # partial add-tree over the already-scaled non-tail branches:
        # q = (r1+r4) + (1x1+r8), in place — executes inside the last
        # gather's window on DVE (tile's Pool queue is blocked until that
        # gather completes)
        ptree = {}
        for mc in range(2):
            pa = y16_all[2][mc]                      # r1 (in place)
            nc.vector.tensor_tensor(pa[:], pa[:], y16_all[3][mc][:],
                                    op=ALU.add)
            pb = y16_all[1][mc]                      # 1x1 (in place)
            nc.vector.tensor_tensor(pb[:], pb[:], y16_all[4][mc][:],
                                    op=ALU.add)
            nc.vector.tensor_tensor(pa[:], pa[:], pb[:], op=ALU.add)
            ptree[mc] = pa

        # drain the tail group (r12): coefs + clip/scale (mc1 first so the
        # Pool-side mc1 chain below starts as early as possible)
        for (when, gg) in pending:
            merge_and_coefs(gg, gathers[gg])
            c = coefs[gg]
            for mc in (1, 0):
                yt = y16_all[5][mc]
                nc.vector.tensor_scalar(yt[:], yt[:], c["lo"][:, 0, mc:mc + 1],
                                        c["hi"][:, 0, mc:mc + 1],
                                        op0=ALU.max, op1=ALU.min)
                nc.vector.tensor_scalar(yt[:], yt[:], c["a"][:, 0, mc:mc + 1],
                                        None, op0=ALU.mult)

        # ---- final: s = q + r12 + s0, store as fp16.  DVE takes mc0, the
        # (post-gather free) Pool takes mc1.
        for mc in (1, 0):
            eng = nc.gpsimd if mc == 1 else nc.vector
            pa = ptree[mc]
            eng.tensor_tensor(pa[:], pa[:], y16_all[5][mc][:], op=ALU.add)
            for t in range(2):
                sf = sbout.tile([P, 2048], F16, tag="sf")
                eng.tensor_scalar(sf[:], pa[:, t * 2048:(t + 1) * 2048],
                                  1.0, s0[:, mc:mc + 1],
                                  op0=ALU.mult, op1=ALU.add)
                deng = nc.sync if t == 0 else nc.scalar
                deng.dma_start(
                    out.ap()[mc * P:(mc + 1) * P].rearrange(
                        "m h w -> m (h w)")[:, t * 2048:(t + 1) * 2048],
                    sf[:])

    nc.compile()
    return nc


def pack_weights(w_pool, w1, w3):
    """Host filter transform: sign -> DoubleRow k-interleave, fp8.

    wt[k, t*2+mc, i, m] = sign(W_t[mc*128+m, i*128+k]); block (t*2+mc) is the
    stationary [2, 128] operand for logical tap t / out-channel chunk mc.
    """
    mats = [np.sign(np.asarray(w_pool, np.float32).reshape(COUT, CIN)),
            np.sign(np.asarray(w1, np.float32).reshape(COUT, CIN))]
    w3 = np.asarray(w3, np.float32)
    for i in range(len(RATES)):
        for ky in range(3):
            for kx in range(3):
                mats.append(np.sign(w3[i, :, :, ky, kx]))
    wt = np.zeros((P, NT * 2, 2, P), np.float32)  # [k, blk, i, m]
    for t, m in enumerate(mats):
        for mc in range(2):
            for i in range(2):
                blk = m[mc * P:(mc + 1) * P, i * P:(i + 1) * P]   # [m, k]
                wt[:, t * 2 + mc, i, :] = blk.T
    return wt.astype(mybir.dt.np(FP8))


def pack_coef(g_pool, b_pool, g1, b1, g3, b3):
    gs = [g_pool, g1] + [g3[i] for i in range(len(RATES))]
    bs = [b_pool, b1] + [b3[i] for i in range(len(RATES))]
    coef = np.zeros((P, 2, 12), np.float32)
    for j in range(6):
        g = np.asarray(gs[j], np.float32)
        b = np.asarray(bs[j], np.float32)
        for mc in range(2):
            coef[:, mc, 2 * j] = g[mc * P:(mc + 1) * P]
            coef[:, mc, 2 * j + 1] = b[mc * P:(mc + 1) * P]
    return coef


_NC = None


def _get_nc():
    global _NC
    if _NC is None:
        _NC = build(N_CORES)
    return _NC


def make_in_maps(x, w_pool, g_pool, b_pool, w1, g1, b1, w3, g3, b3):
    x = np.asarray(x, np.float32)
    wt = pack_weights(w_pool, w1, w3)
    coef = pack_coef(g_pool, b_pool, g1, b1, g3, b3)
    return [
        {"xs": np.ascontiguousarray(x[c]), "wt": wt, "coef": coef}
        for c in range(x.shape[0])
    ]


def kernel(x, w_pool, g_pool, b_pool, w1, g1, b1, w3, g3, b3):
    nc = _get_nc()
    in_maps = make_in_maps(x, w_pool, g_pool, b_pool, w1, g1, b1, w3, g3, b3)
    res = run_bass_kernel_spmd(nc, in_maps, core_ids=list(range(N_CORES)))
    return np.stack([res.results[c]["out"] for c in range(N_CORES)],
                    axis=0).astype(np.float32)
